# revision 1
# baseline (speedup 1.0000x reference)
"""Distributed multi-head attention forward for 8 TRN2 NeuronCores.

Problem: y = proj(softmax((x Wq^T + bq)(x Wk^T + bk)^T / sqrt(hd)) (x Wv^T + bv))
  x: [4, 2048, 1024], 16 heads, head_dim 64, fp32.

Sharding: query-parallel with redundant K/V. Core i owns global flat tokens
[i*1024, (i+1)*1024) as queries (cores 2b, 2b+1 own batch b). Each core
receives the FULL batch activations (its own tokens first, partner's second —
host arranges this so the SPMD graph is core-independent), computes K/V for
all 2048 batch tokens locally (+50% projection FLOPs), Q for its own 1024,
then full 16-head attention for its query slice and the output projection.
No collectives at all; output is token-sharded and concatenated on the host.
(K/V see partner tokens in a consistent order; softmax is permutation-
invariant over keys so the result is exact.)

Layouts (host pre-transposes, all free):
  xT      [D, 2*tq]  feature-major activations (my tokens | partner tokens)
  w_{q,k}T[D, D]     so qT/kT come out feature-major: qT[f, t]
  w_vT    [D, D]     v computed token-major: v[t, f]
  bias    [128, D/128] partition-major (per-partition scalars for ACT bias)
  w_projT [D, D]
  out     yT [D, tq] (host transposes back)

b_v is folded into b_proj on the host: (o + bv) Wp^T + bp = o Wp^T + (Wp bv + bp).

K goes through DRAM as kT [D, 2048] (contiguous per-head-pair rows).
V goes through DRAM head-blocked: vbuf[kt, p, h*65:(h+1)*65] where cols
0:64 of each 65-block are v values for head h, col 64 is a baked-in 1.0
(ones column). The AV matmul then computes both the attention output and
the softmax denominator in one accumulation:
  OT_aug[0:64, q] = sum_k v[k, d] p[k, q]   OT_aug[64, q] = sum_k p[k, q] = Z
Softmax skips the max subtraction (scores ~N(0, 0.17), exp safe in fp32).
Normalization: Z row -> SBUF, reciprocal_approx_fast, gpsimd
partition_broadcast to [64, qch], one fused DVE multiply (PSUM-evict+scale).

All matmul operands are float32r (fp32 bits, PE rounds internally; full
PE rate at free-dim >= 256, ~1.6e-4 rel err vs 4x-slower exact fp32).
"""

import numpy as np

P = 128
D = 1024
NH = 16
HD = 64
SCALE = 1.0 / float(np.sqrt(HD))
NCORES = 8
TQ = 1024          # query tokens per core
B, T = 4, 2048

_COMPILED = {}


def _full_cfg():
    return dict(D=D, NH=NH, TQ=TQ, n_devices=NCORES)


def build(cfg=None):
    """Build + compile the per-core Bass graph. Returns the compiled Bacc."""
    from concourse import bacc
    import concourse.mybir as mybir
    import concourse.tile as tile

    if cfg is None:
        cfg = _full_cfg()
    d = cfg["D"]; nh = cfg["NH"]; tq = cfg["TQ"]
    n_dev = cfg["n_devices"]
    tk = 2 * tq                      # batch tokens for k/v
    f32 = mybir.dt.float32
    bf16 = mybir.dt.bfloat16
    AF = mybir.ActivationFunctionType

    nft = d // P                     # feature tiles (also contraction chunks)
    qch = min(512, tq)               # q free-dim chunk
    nqc = tq // qch
    kch = min(512, tq)               # token chunk for k projection
    nkc = tk // kch
    nkt = tk // P                    # k tiles along batch tokens
    nhp = nh // 2                    # head pairs
    fch = min(512, d)
    nfc = d // fch
    hpf = fch // HD                  # heads per v f-chunk

    nc = bacc.Bacc("TRN2", target_bir_lowering=False, debug=False,
                   num_devices=n_dev)

    xT = nc.dram_tensor("xT", [d, tk], bf16, kind="ExternalInput")
    w_qT = nc.dram_tensor("w_qT", [d, d], bf16, kind="ExternalInput")
    w_kT = nc.dram_tensor("w_kT", [d, d], bf16, kind="ExternalInput")
    w_vT = nc.dram_tensor("w_vT", [d, d], bf16, kind="ExternalInput")
    w_pT = nc.dram_tensor("w_pT", [d, d], bf16, kind="ExternalInput")
    b_q = nc.dram_tensor("b_q", [P, nft], f32, kind="ExternalInput")
    b_k = nc.dram_tensor("b_k", [P, nft], f32, kind="ExternalInput")
    b_p = nc.dram_tensor("b_p", [P, nft], f32, kind="ExternalInput")
    outT = nc.dram_tensor("out", [d, tq], f32, kind="ExternalOutput")

    with tile.TileContext(nc) as tc:
        with (
            tc.tile_pool(name="persist", bufs=1) as persist,
            tc.tile_pool(name="bias", bufs=1) as biasp,
            tc.tile_pool(name="xpool", bufs=2) as xpool,
            tc.tile_pool(name="wpool", bufs=2) as wpool,
            tc.tile_pool(name="ptile", bufs=3) as ptile,
            tc.tile_pool(name="zpool", bufs=1) as zpool,
            tc.tile_pool(name="rzbp", bufs=1) as rzbp,
            tc.tile_pool(name="ypool", bufs=1) as ypool,
            tc.tile_pool(name="psmm", bufs=2, space="PSUM") as psmm,
            tc.tile_pool(name="pst", bufs=2, space="PSUM") as pst,
            tc.tile_pool(name="pot", bufs=2, space="PSUM") as pot,
        ):
            # ---- persistent SBUF ----
            q_all = persist.tile([P, nft, tq], bf16)     # qT, feature-major
            kt_all = persist.tile([P, nhp, tk], bf16)    # kT by head pair
            vt_all = persist.tile([P, nkt, nh * (HD + 1)], bf16)  # v + ones col
            ot_all = persist.tile([P, nft, tq], bf16)    # attention out^T
            wp_sb = persist.tile([P, nft, d], bf16)
            bq_sb = biasp.tile([P, nft], f32)
            nc.sync.dma_start(bq_sb[:], b_q[:])
            bk_sb = biasp.tile([P, nft], f32)
            nc.sync.dma_start(bk_sb[:], b_k[:])
            bp_sb = biasp.tile([P, nft], f32)
            nc.sync.dma_start(bp_sb[:], b_p[:])

            # x for both batch halves, feature-major, chunked by d so the
            # first matmuls can start before the whole input has landed
            xh = []
            for half in range(tk // tq):
                x_sb = xpool.tile([P, nft, tq], bf16, tag="x")
                for dc in range(nft):
                    nc.sync.dma_start(
                        x_sb[:, dc, :],
                        xT[dc * P:(dc + 1) * P, half * tq:(half + 1) * tq])
                xh.append(x_sb)

            # ones columns for the AV sum-of-exp trick
            nc.vector.memset(vt_all[:].rearrange("p k (h e) -> p k h e",
                                                 e=HD + 1)[:, :, :, HD], 1.0)

            # ---- projection work units (k/q/v/out-proj), emitted
            # interleaved with attention pairs so projection matmuls fill
            # the PE gaps in the ACT(exp)-bound attention stream
            wk_sb = wpool.tile([P, nft, d], bf16, tag="w", bufs=3)
            for dc in range(nft):
                nc.sync.dma_start(wk_sb[:, dc, :], w_kT[dc * P:(dc + 1) * P, :])
            wv_sb = wpool.tile([P, nft, d], bf16, tag="w", bufs=3)
            for dc in range(nft):
                nc.sync.dma_start(wv_sb[:, dc, :], w_vT[dc * P:(dc + 1) * P, :])
            wq_sb = wpool.tile([P, nft, d], bf16, tag="w", bufs=3)
            for dc in range(nft):
                nc.sync.dma_start(wq_sb[:, dc, :], w_qT[dc * P:(dc + 1) * P, :])
            for dc in range(nft):
                nc.sync.dma_start(wp_sb[:, dc, :], w_pT[dc * P:(dc + 1) * P, :])

            def k_unit(ft, c):
                xsb = xh[c * kch // tq]
                t0 = (c * kch) % tq
                ps = psmm.tile([P, kch], f32, tag="mm", name="ps_k")
                for dc in range(nft):
                    nc.tensor.matmul(
                        ps[:],
                        wk_sb[:, dc, ft * P:(ft + 1) * P],
                        xsb[:, dc, t0:t0 + kch],
                        start=(dc == 0), stop=(dc == nft - 1))
                nc.vector.tensor_scalar_add(
                    kt_all[:, ft, c * kch:(c + 1) * kch], ps[:],
                    bk_sb[:, ft:ft + 1])

            def q_unit(ft, c):
                ps = psmm.tile([P, qch], f32, tag="mm", name="ps_q")
                for dc in range(nft):
                    nc.tensor.matmul(
                        ps[:],
                        wq_sb[:, dc, ft * P:(ft + 1) * P],
                        xh[0][:, dc, c * qch:(c + 1) * qch],
                        start=(dc == 0), stop=(dc == nft - 1))
                nc.vector.tensor_scalar_add(
                    q_all[:, ft, c * qch:(c + 1) * qch], ps[:],
                    bq_sb[:, ft:ft + 1])

            def v_unit(fc, tt):
                xsb = xh[tt * P // tq]
                tcol = (tt * P) % tq
                ps = psmm.tile([P, fch], f32, tag="mm", name="ps_v")
                for dc in range(nft):
                    nc.tensor.matmul(
                        ps[:],
                        xsb[:, dc, tcol:tcol + P],
                        wv_sb[:, dc, fc * fch:(fc + 1) * fch],
                        start=(dc == 0), stop=(dc == nft - 1))
                dst = (vt_all[:, tt, fc * hpf * (HD + 1):(fc + 1) * hpf * (HD + 1)]
                       .rearrange("p (h e) -> p h e", e=HD + 1)[:, :, 0:HD])
                nc.vector.tensor_copy(
                    dst, ps[:].rearrange("p (h e) -> p h e", e=HD))

            def proj_unit(qc, jt):
                ps = psmm.tile([P, qch], f32, tag="mm", name="ps_p")
                for dc in range(nft):
                    nc.tensor.matmul(
                        ps[:],
                        wp_sb[:, dc, jt * P:(jt + 1) * P],
                        ot_all[:, dc, qc * qch:(qc + 1) * qch],
                        start=(dc == 0), stop=(dc == nft - 1))
                ysb = ypool.tile([P, qch], f32, name="ysb")
                nc.vector.tensor_scalar_add(ysb[:], ps[:], bp_sb[:, jt:jt + 1])
                nc.sync.dma_start(
                    outT[jt * P:(jt + 1) * P, qc * qch:(qc + 1) * qch],
                    ysb[:])

            def attn_pair(hp, qc):
                hA, hB = 2 * hp, 2 * hp + 1
                otA = pot.tile([P, qch], f32, tag="ot", name="otA")
                otB = pot.tile([P, qch], f32, tag="ot", name="otB")
                qA = q_all[0:HD, hp, qc * qch:(qc + 1) * qch]
                qB = q_all[HD:2 * HD, hp, qc * qch:(qc + 1) * qch]

                def emit_av(k, pt):
                    for (ot, h, p0) in ((otA, hA, 0), (otB, hB, qch)):
                        nc.tensor.matmul(
                            ot[0:HD + 1, :],
                            vt_all[:, k, h * (HD + 1):(h + 1) * (HD + 1)],
                            pt[:, p0:p0 + qch],
                            start=(k == 0), stop=(k == nkt - 1))

                # AV emitted two k-tiles behind its exp (pt pool holds 3)
                # so the in-order PE queue never waits on the ACT stream
                pend = []
                for k in range(nkt):
                    st = pst.tile([P, 2 * qch], f32, tag="st", name="st")
                    nc.tensor.matmul(
                        st[:, 0:qch],
                        kt_all[0:HD, hp, k * P:(k + 1) * P],
                        qA, start=True, stop=True)
                    nc.tensor.matmul(
                        st[:, qch:2 * qch],
                        kt_all[HD:2 * HD, hp, k * P:(k + 1) * P],
                        qB, start=True, stop=True)
                    pt = ptile.tile([P, 2 * qch], bf16, tag="pt", name="pt")
                    nc.scalar.activation(pt[:], st[:], AF.Exp, scale=SCALE)
                    pend.append((k, pt))
                    if len(pend) > 2:
                        emit_av(*pend.pop(0))
                for pe_ in pend:
                    emit_av(*pe_)

                for (ot, hh) in ((otA, 0), (otB, 1)):
                    zrow = zpool.tile([1, qch], f32, tag="z", name="zrow")
                    nc.vector.tensor_copy(zrow[:], ot[HD:HD + 1, :])
                    rz = zpool.tile([1, qch], f32, tag="z2", name="rz")
                    nc.vector.reciprocal_approx_fast(rz[:], zrow[:])
                    rzb = rzbp.tile([HD, qch], f32, name="rzb")
                    nc.gpsimd.partition_broadcast(rzb[:], rz[:])
                    nc.vector.tensor_mul(
                        ot_all[hh * HD:(hh + 1) * HD, hp,
                               qc * qch:(qc + 1) * qch],
                        ot[0:HD, :], rzb[:])

            # ---- the interleaved schedule ----
            # kq(ft) and v(fc) units ahead of the attention pairs that need
            # them; later projection units slot between attention pairs
            def kq(ft):
                for c in range(nkc):
                    k_unit(ft, c)
                for c in range(nqc):
                    q_unit(ft, c)

            kq(0)
            for tt in range(nkt):
                v_unit(0, tt)
            attn_pair(0, 0)
            if nqc > 1:
                attn_pair(0, 1)
            if nhp > 1:
                kq(1)

            # filler queues: remaining v chunks and k/q feature tiles.
            # Required units are force-drained before the pair that reads
            # them; otherwise one filler is emitted after each pair to keep
            # PE fed during the ACT-bound attention stream.
            pending_v = [(fc, tt) for fc in range(1, nfc) for tt in range(nkt)]
            pending_kq = list(range(2, nhp))

            def emit_required(hp):
                need_fc = (2 * hp) // hpf
                while pending_kq and pending_kq[0] <= hp:
                    kq(pending_kq.pop(0))
                while pending_v and pending_v[0][0] <= need_fc:
                    fc, tt = pending_v.pop(0)
                    v_unit(fc, tt)

            def emit_filler(n):
                for _ in range(n):
                    if pending_v:
                        fc, tt = pending_v.pop(0)
                        v_unit(fc, tt)
                    elif pending_kq:
                        kq(pending_kq.pop(0))

            pairs = [(hp, qc) for hp in range(1, nhp) for qc in range(nqc)]
            nfill = len(pending_v) + 2 * len(pending_kq)
            per = max(1, (nfill + len(pairs) - 1) // max(1, len(pairs)))
            for (hp, qc) in pairs:
                emit_required(hp)
                attn_pair(hp, qc)
                emit_filler(per)
                if nqc > 1 and hp == nhp - 1 and qc == 0:
                    emit_filler(len(pending_v) + len(pending_kq))
                    for jt in range(nft):
                        proj_unit(0, jt)
            emit_filler(len(pending_v) + len(pending_kq))
            if nqc > 1:
                for jt in range(nft):
                    proj_unit(1, jt)
            else:
                for jt in range(nft):
                    proj_unit(0, jt)

    nc.compile()
    return nc


def make_in_maps(inputs, cfg=None):
    """Host-side sharding: full inputs -> per-core input dicts."""
    if cfg is None:
        cfg = _full_cfg()
    d = cfg["D"]; tq = cfg["TQ"]; n_dev = cfg["n_devices"]; nh = cfg["NH"]
    nft = d // P
    nkt = 2 * tq // P

    x = np.asarray(inputs["x"], dtype=np.float32)
    w_qkv = np.asarray(inputs["w_qkv"], dtype=np.float32)
    b_qkv = np.asarray(inputs["b_qkv"], dtype=np.float32)
    w_proj = np.asarray(inputs["w_proj"], dtype=np.float32)
    b_proj = np.asarray(inputs["b_proj"], dtype=np.float32)

    import ml_dtypes
    bf = ml_dtypes.bfloat16

    x_flat = x.reshape(-1, d)
    w_qT = np.ascontiguousarray(w_qkv[0:d].T).astype(bf)
    w_kT = np.ascontiguousarray(w_qkv[d:2 * d].T).astype(bf)
    w_vT = np.ascontiguousarray(w_qkv[2 * d:3 * d].T).astype(bf)
    b_q = b_qkv[0:d]; b_k = b_qkv[d:2 * d]; b_v = b_qkv[2 * d:3 * d]
    w_pT = np.ascontiguousarray(w_proj.T).astype(bf)
    b_p_eff = b_proj + w_proj @ b_v

    def bias_tile(b):
        return np.ascontiguousarray(b.reshape(nft, P).T)

    shared = {
        "w_qT": w_qT, "w_kT": w_kT, "w_vT": w_vT, "w_pT": w_pT,
        "b_q": bias_tile(b_q), "b_k": bias_tile(b_k), "b_p": bias_tile(b_p_eff),
    }
    in_maps = []
    for i in range(n_dev):
        mine = x_flat[i * tq:(i + 1) * tq]
        partner = x_flat[(i ^ 1) * tq:((i ^ 1) + 1) * tq]
        xT_i = np.ascontiguousarray(
            np.concatenate([mine, partner], axis=0).T).astype(bf)
        in_maps.append({"xT": xT_i, **shared})
    return in_maps


def assemble_output(results, inputs, cfg=None):
    if cfg is None:
        cfg = _full_cfg()
    d = cfg["D"]; tq = cfg["TQ"]; n_dev = cfg["n_devices"]
    x = np.asarray(inputs["x"])
    y = np.empty((n_dev * tq, d), dtype=np.float32)
    for i in range(n_dev):
        y[i * tq:(i + 1) * tq] = results[i]["out"].T
    return y.reshape(x.shape)


def run(inputs, trace=False, **kw):
    from concourse.bass_utils import run_bass_kernel_spmd
    key = "full"
    if key not in _COMPILED:
        _COMPILED[key] = build()
    nc = _COMPILED[key]
    in_maps = make_in_maps(inputs)
    res = run_bass_kernel_spmd(nc, in_maps, core_ids=list(range(NCORES)),
                               trace=trace, **kw)
    return res


def kernel(**inputs) -> np.ndarray:
    res = run(inputs, trace=False)
    return assemble_output(res.results, inputs)



# revision 12
# speedup vs baseline: 1.0961x; 1.0961x over previous
"""Distributed multi-head attention forward for 8 TRN2 NeuronCores.

Problem: y = proj(softmax((x Wq^T + bq)(x Wk^T + bk)^T / sqrt(hd)) (x Wv^T + bv))
  x: [4, 2048, 1024], 16 heads, head_dim 64, fp32.

Sharding: query-parallel with redundant K/V (no collectives). Core i owns
global flat tokens [i*1024, (i+1)*1024) as queries; each core receives the
full batch activations (own tokens first, partner's second), computes K/V
for all 2048 batch tokens, Q for its own 1024, then full 16-head attention
for its query slice and the output projection. Output is token-sharded and
concatenated on the host.

v2 speedups over the 490us baseline:
  * AV matmuls in fp8e4 DoubleRow (2 k-tiles per matmul) for 11 of 16
    k-tiles; v and probs stored e4m3. Remaining 5 k-tiles stay bf16.
  * exp split across two engines: ACT does 11/16 k-tiles (table exp, fp8
    out), DVE does 5/16 via a Schraudolph bit-trick (probs = int16
    round(A*s+B) reinterpreted as bf16, one tensor_scalar per tile).
  * attention starts at ~15us (vs 70): per-chunk k/q/v units emitted just
    in time, projections interleave as fillers inside the ACT-bound
    attention stream, O-proj(qc=0) fills the second half via a
    qc-interleaved pair order.
  * all activations/weights scaled by 16 on the host so fp8 ranges are
    safe; output descaled by 256 on the host.

b_v is folded into b_proj on the host: (o + bv) Wp^T + bp = o Wp^T + (Wp bv + bp).
"""

import numpy as np

P = 128
D = 1024
NH = 16
HD = 64
SCALE = 1.0 / float(np.sqrt(HD))
NCORES = 8
TQ = 1024          # query tokens per core
B, T = 4, 2048
TK = 2048          # batch tokens for k/v

NFT = D // P       # feature tiles (8)
QCH = 512          # q free-dim chunk
NQC = TQ // QCH    # 2
KCH = 512          # token chunk for k projection
NKC = TK // KCH    # 4
NKT = TK // P      # k tiles along batch tokens (16)
NKP = NKT // 2     # k-tile pairs (8)
NHP = NH // 2      # head pairs (8)
FCH = 512          # v feature chunk
NFC = D // FCH     # 2
HPF = FCH // HD    # heads per v f-chunk (8)

S_W = 16.0         # weight/activation scale for fp8 ranges
OUT_SCALE = S_W * S_W   # output descale factor (host divides)

# k-tiles whose exp runs on DVE (Schraudolph) with bf16 probs + bf16 v.
# Chosen so the fp8 k-tiles pair up on even boundaries for DoubleRow:
# fp8 tiles {0,1,2,3,6,7,8,9,12,13,14}: DR pairs (0,1),(2,3),(6,7),(8,9),
# (12,13); tile 14 runs fp8 normal-mode; tiles 4,5,10,11,15 run bf16.
SCH_KT = (4, 5, 10, 11, 15)
SCH_IDX = {t: i for i, t in enumerate(SCH_KT)}

# Schraudolph constants: st holds 256*s_true, probs = exp(st/2048).
# bf16 bit pattern of exp2(y) ~ int16(128*(y + 127 - c)).
SCH_C = 0.04367
SCH_A = 128.0 * float(np.log2(np.e)) / 2048.0
SCH_B = 128.0 * (127.0 - SCH_C)
EXP_SCALE = SCALE / (S_W * S_W)   # activation scale for table exp

USE_SCH = True    # Schraudolph bit-trick exp on DVE for SCH_KT tiles
USE_DR = True     # fp8 DoubleRow AV matmuls

_COMPILED = {}


def build():
    """Build + compile the per-core Bass graph. Returns the compiled Bacc."""
    from concourse import bacc
    import concourse.mybir as mybir
    import concourse.tile as tile

    f32 = mybir.dt.float32
    bf16 = mybir.dt.bfloat16
    f8 = mybir.dt.float8e4
    i16 = mybir.dt.int16
    AF = mybir.ActivationFunctionType
    DR = mybir.MatmulPerfMode.DoubleRow

    nc = bacc.Bacc("TRN2", target_bir_lowering=False, debug=False,
                   num_devices=NCORES)

    xT = nc.dram_tensor("xT", [D, TK], bf16, kind="ExternalInput")
    w_qT = nc.dram_tensor("w_qT", [D, D], bf16, kind="ExternalInput")
    w_kT = nc.dram_tensor("w_kT", [D, D], bf16, kind="ExternalInput")
    w_vT = nc.dram_tensor("w_vT", [D, D], bf16, kind="ExternalInput")
    w_pT = nc.dram_tensor("w_pT", [D, D], bf16, kind="ExternalInput")
    b_q = nc.dram_tensor("b_q", [P, NFT], f32, kind="ExternalInput")
    b_k = nc.dram_tensor("b_k", [P, NFT], f32, kind="ExternalInput")
    b_p = nc.dram_tensor("b_p", [P, NFT], f32, kind="ExternalInput")
    outT = nc.dram_tensor("out", [D, TQ], f32, kind="ExternalOutput")

    NSCH = len(SCH_KT)

    with tile.TileContext(nc) as tc:
        with (
            tc.tile_pool(name="persist", bufs=1) as persist,
            tc.tile_pool(name="bias", bufs=1) as biasp,
            tc.tile_pool(name="pt8p", bufs=2) as pt8p,
            tc.tile_pool(name="ptbp", bufs=2) as ptbp,
            tc.tile_pool(name="zpool", bufs=1) as zpool,
            tc.tile_pool(name="rzbp", bufs=1) as rzbp,
            tc.tile_pool(name="ypool", bufs=1) as ypool,
            tc.tile_pool(name="psmm", bufs=2, space="PSUM") as psmm,
            tc.tile_pool(name="pst", bufs=2, space="PSUM") as pst,
            tc.tile_pool(name="pot", bufs=2, space="PSUM") as pot,
        ):
            # ---- persistent SBUF ----
            x_sb = persist.tile([P, NFT, TK], bf16)       # 32KB/part
            wk_sb = persist.tile([P, NFT, D], bf16)       # 16KB
            wq_sb = persist.tile([P, NFT, D], bf16)       # 16KB
            wv_sb = persist.tile([P, NFT, D], bf16)       # 16KB
            wp_sb = persist.tile([P, NFT, D], bf16)       # 16KB
            q_all = persist.tile([P, NHP, TQ], bf16)      # 16KB
            kt_all = persist.tile([P, NHP, TK], bf16)     # 32KB
            vt8 = persist.tile([P, NKT, NH * (HD + 1)], f8)     # 16.25KB
            vtb = persist.tile([P, NSCH, NH * (HD + 1)], bf16)  # 10.2KB
            ot_all = persist.tile([P, NFT, TQ], bf16)     # 16KB

            bq_sb = biasp.tile([P, NFT], f32)
            bk_sb = biasp.tile([P, NFT], f32)
            bp_sb = biasp.tile([P, NFT], f32)
            nc.sync.dma_start(bq_sb[:], b_q[:])
            nc.sync.dma_start(bk_sb[:], b_k[:])
            nc.sync.dma_start(bp_sb[:], b_p[:])

            # DMA priority order: wk, x half0, wq, x half1, wv, wp.
            # Attention pair 0's first exps need only wk + x-half0 + wq.
            for dc in range(NFT):
                nc.sync.dma_start(wk_sb[:, dc, :], w_kT[dc * P:(dc + 1) * P, :])
            for dc in range(NFT):
                nc.sync.dma_start(x_sb[:, dc, 0:TQ],
                                  xT[dc * P:(dc + 1) * P, 0:TQ])
            for dc in range(NFT):
                nc.sync.dma_start(wq_sb[:, dc, :], w_qT[dc * P:(dc + 1) * P, :])
            for dc in range(NFT):
                nc.sync.dma_start(x_sb[:, dc, TQ:TK],
                                  xT[dc * P:(dc + 1) * P, TQ:TK])
            for dc in range(NFT):
                nc.sync.dma_start(wv_sb[:, dc, :], w_vT[dc * P:(dc + 1) * P, :])
            for dc in range(NFT):
                nc.sync.dma_start(wp_sb[:, dc, :], w_pT[dc * P:(dc + 1) * P, :])

            # ones columns for the AV sum-of-exp trick (both v stores)
            nc.vector.memset(vt8[:].rearrange("p k (h e) -> p k h e",
                                              e=HD + 1)[:, :, :, HD], 1.0)
            nc.vector.memset(vtb[:].rearrange("p k (h e) -> p k h e",
                                              e=HD + 1)[:, :, :, HD], 1.0)

            # ---- projection work units ----
            def k_unit(ft, c):
                ps = psmm.tile([P, KCH], f32, tag="mm", name="ps_k")
                for dc in range(NFT):
                    nc.tensor.matmul(
                        ps[:],
                        wk_sb[:, dc, ft * P:(ft + 1) * P],
                        x_sb[:, dc, c * KCH:(c + 1) * KCH],
                        start=(dc == 0), stop=(dc == NFT - 1))
                nc.vector.tensor_scalar_add(
                    kt_all[:, ft, c * KCH:(c + 1) * KCH], ps[:],
                    bk_sb[:, ft:ft + 1])

            def q_unit(ft, qc):
                ps = psmm.tile([P, QCH], f32, tag="mm", name="ps_q")
                for dc in range(NFT):
                    nc.tensor.matmul(
                        ps[:],
                        wq_sb[:, dc, ft * P:(ft + 1) * P],
                        x_sb[:, dc, qc * QCH:(qc + 1) * QCH],
                        start=(dc == 0), stop=(dc == NFT - 1))
                nc.vector.tensor_scalar_add(
                    q_all[:, ft, qc * QCH:(qc + 1) * QCH], ps[:],
                    bq_sb[:, ft:ft + 1])

            def v_unit(fc, tt):
                ps = psmm.tile([P, FCH], f32, tag="mm", name="ps_v")
                for dc in range(NFT):
                    nc.tensor.matmul(
                        ps[:],
                        x_sb[:, dc, tt * P:(tt + 1) * P],
                        wv_sb[:, dc, fc * FCH:(fc + 1) * FCH],
                        start=(dc == 0), stop=(dc == NFT - 1))
                if tt in SCH_IDX:
                    vdst = vtb[:, SCH_IDX[tt],
                               fc * HPF * (HD + 1):(fc + 1) * HPF * (HD + 1)]
                else:
                    vdst = vt8[:, tt,
                               fc * HPF * (HD + 1):(fc + 1) * HPF * (HD + 1)]
                nc.vector.tensor_copy(
                    vdst.rearrange("p (h e) -> p h e", e=HD + 1)[:, :, 0:HD],
                    ps[:].rearrange("p (h e) -> p h e", e=HD))

            def o_unit(qc, jt):
                ps = psmm.tile([P, QCH], f32, tag="mm", name="ps_p")
                for dc in range(NFT):
                    nc.tensor.matmul(
                        ps[:],
                        wp_sb[:, dc, jt * P:(jt + 1) * P],
                        ot_all[:, dc, qc * QCH:(qc + 1) * QCH],
                        start=(dc == 0), stop=(dc == NFT - 1))
                ysb = ypool.tile([P, QCH], f32, name="ysb")
                nc.vector.tensor_scalar_add(ysb[:], ps[:], bp_sb[:, jt:jt + 1])
                nc.sync.dma_start(
                    outT[jt * P:(jt + 1) * P, qc * QCH:(qc + 1) * QCH],
                    ysb[:])

            # ---- attention pair ----
            def attn_pair(hp, qc, hook):
                hA, hB = 2 * hp, 2 * hp + 1
                otA = pot.tile([P, QCH], f32, tag="ot", name="otA")
                otB = pot.tile([P, QCH], f32, tag="ot", name="otB")
                qA = q_all[0:HD, hp, qc * QCH:(qc + 1) * QCH]
                qB = q_all[HD:2 * HD, hp, qc * QCH:(qc + 1) * QCH]

                def emit_av(g, t8, tb):
                    last = (g == NKP - 1)
                    for (ot, h, hi) in ((otA, hA, 0), (otB, hB, 1)):
                        hs = slice(h * (HD + 1), (h + 1) * (HD + 1))
                        if t8 is not None and tb is None:
                            if USE_DR:
                                # full DoubleRow pair (2 k-tiles, one matmul)
                                nc.tensor.matmul(
                                    ot[0:HD + 1, :],
                                    vt8[:, 2 * g:2 * g + 2, hs],
                                    t8[:, 0:2, hi, :],
                                    start=(g == 0), stop=last,
                                    perf_mode=DR)
                            else:
                                for kk in range(2):
                                    nc.tensor.matmul(
                                        ot[0:HD + 1, :],
                                        vt8[:, 2 * g + kk, hs],
                                        t8[:, kk, hi, :],
                                        start=(g == 0 and kk == 0),
                                        stop=(last and kk == 1))
                        elif t8 is None:
                            # both k-tiles bf16
                            for kk in range(2):
                                m = SCH_IDX[2 * g + kk]
                                nc.tensor.matmul(
                                    ot[0:HD + 1, :],
                                    vtb[:, m, hs],
                                    tb[:, kk, hi, :],
                                    start=False, stop=(last and kk == 1))
                        else:
                            # mixed: k-tile 2g fp8 normal-mode, 2g+1 bf16
                            nc.tensor.matmul(
                                ot[0:HD + 1, :],
                                vt8[:, 2 * g, hs],
                                t8[:, 0, hi, :],
                                start=False, stop=False)
                            m = SCH_IDX[2 * g + 1]
                            nc.tensor.matmul(
                                ot[0:HD + 1, :],
                                vtb[:, m, hs],
                                tb[:, 1, hi, :],
                                start=False, stop=last)

                pend = []
                for g in range(NKP):
                    n8 = sum(1 for kk in range(2) if (2 * g + kk) not in SCH_IDX)
                    t8 = pt8p.tile([P, 2, 2, QCH], f8, tag="pt8",
                                   name="pt8") if n8 else None
                    tb = ptbp.tile([P, 2, 2, QCH], bf16, tag="ptb",
                                   name="ptb") if n8 < 2 else None
                    for kk in range(2):
                        k = 2 * g + kk
                        st = pst.tile([P, 2 * QCH], f32, tag="st", name="st")
                        nc.tensor.matmul(
                            st[:, 0:QCH],
                            kt_all[0:HD, hp, k * P:(k + 1) * P],
                            qA, start=True, stop=True)
                        nc.tensor.matmul(
                            st[:, QCH:2 * QCH],
                            kt_all[HD:2 * HD, hp, k * P:(k + 1) * P],
                            qB, start=True, stop=True)
                        if k in SCH_IDX:
                            dst = tb[:, kk].rearrange("p a b -> p (a b)")
                            if USE_SCH:
                                nc.vector.tensor_scalar(
                                    dst.bitcast(i16), st[:],
                                    SCH_A, SCH_B,
                                    mybir.AluOpType.mult, mybir.AluOpType.add)
                            else:
                                nc.scalar.activation(dst, st[:], AF.Exp,
                                                     scale=EXP_SCALE)
                        else:
                            dst = t8[:, kk].rearrange("p a b -> p (a b)")
                            nc.scalar.activation(dst, st[:], AF.Exp,
                                                 scale=EXP_SCALE)
                    hook(g)
                    pend.append((g, t8, tb))
                    if len(pend) > 1:
                        emit_av(*pend.pop(0))
                for e in pend:
                    emit_av(*e)

                for (ot, hh) in ((otA, 0), (otB, 1)):
                    zrow = zpool.tile([1, QCH], f32, tag="zr", name="zrow")
                    nc.vector.tensor_copy(zrow[:], ot[HD:HD + 1, :])
                    rz = zpool.tile([1, QCH], f32, tag="z", name="rz")
                    nc.vector.reciprocal_approx_fast(rz[:], zrow[:])
                    rzb = rzbp.tile([HD, QCH], f32, name="rzb")
                    nc.gpsimd.partition_broadcast(rzb[:], rz[:])
                    nc.vector.tensor_mul(
                        ot_all[hh * HD:(hh + 1) * HD, hp,
                               qc * QCH:(qc + 1) * QCH],
                        ot[0:HD, :], rzb[:])

            # ---- schedule ----
            # prefix: minimal prereqs of pair (0,0)
            for c in range(NKC):
                k_unit(0, c)
            q_unit(0, 0)
            for tt in range(3):
                v_unit(0, tt)

            # v(fc0) queue drained inside pair (0,0) just ahead of its AV
            vq0 = [(0, tt) for tt in range(3, NKT)]
            # ordered filler queue with prereq markers
            fillers = []
            markers = {}

            def kq(hp):
                return ([("k", hp, c) for c in range(NKC)] + [("q", hp, 0)])

            fillers += kq(1); markers[(1, 0)] = len(fillers)
            fillers += [("q", 0, 1)]; markers[(0, 1)] = len(fillers)
            fillers += [("v", 1, tt) for tt in range(4)]
            fillers += kq(2); markers[(2, 0)] = len(fillers)
            fillers += [("v", 1, tt) for tt in range(4, 8)]
            fillers += kq(3); markers[(3, 0)] = len(fillers)
            fillers += [("q", 1, 1)]; markers[(1, 1)] = len(fillers)
            fillers += [("v", 1, tt) for tt in range(8, NKT)]
            fillers += kq(4); markers[(4, 0)] = len(fillers)
            fillers += [("q", 2, 1)]; markers[(2, 1)] = len(fillers)
            fillers += kq(5); markers[(5, 0)] = len(fillers)
            fillers += [("q", 3, 1)]; markers[(3, 1)] = len(fillers)
            fillers += kq(6); markers[(6, 0)] = len(fillers)
            fillers += kq(7); markers[(7, 0)] = len(fillers)
            fillers += [("q", 4, 1)]; markers[(4, 1)] = len(fillers)
            fillers += [("q", 5, 1)]; markers[(5, 1)] = len(fillers)
            fillers += [("q", 6, 1)]; markers[(6, 1)] = len(fillers)
            fillers += [("q", 7, 1)]; markers[(7, 1)] = len(fillers)

            def emit_unit(u):
                kind = u[0]
                if kind == "k":
                    k_unit(u[1], u[2])
                elif kind == "q":
                    q_unit(u[1], u[2])
                elif kind == "v":
                    v_unit(u[1], u[2])
                else:
                    o_unit(u[1], u[2])

            state = {"fi": 0}

            def drain_to(mark):
                while state["fi"] < mark:
                    emit_unit(fillers[state["fi"]])
                    state["fi"] += 1

            def pop_filler(n):
                for _ in range(n):
                    if state["fi"] < len(fillers):
                        emit_unit(fillers[state["fi"]])
                        state["fi"] += 1

            SEQ = [(0, 0), (1, 0), (0, 1), (2, 0), (3, 0), (1, 1),
                   (4, 0), (2, 1), (5, 0), (3, 1), (6, 0), (7, 0),
                   (4, 1), (5, 1), (6, 1), (7, 1)]

            for (hp, qc) in SEQ:
                if (hp, qc) in markers:
                    drain_to(markers[(hp, qc)])

                if (hp, qc) == (0, 0):
                    def hook(g):
                        # keep v(fc0) two k-tile-pairs ahead of the AV lag
                        while vq0 and vq0[0][1] <= 2 * g + 3:
                            v_unit(*vq0.pop(0))
                else:
                    def hook(g):
                        pop_filler(1)
                attn_pair(hp, qc, hook)
                if (hp, qc) == (0, 0):
                    while vq0:
                        v_unit(*vq0.pop(0))
                if (hp, qc) == (7, 0):
                    fillers.extend(("o", 0, jt) for jt in range(NFT))

            pop_filler(len(fillers))
            for jt in range(NFT):
                o_unit(1, jt)

    nc.compile()
    return nc


def make_in_maps(inputs):
    """Host-side sharding: full inputs -> per-core input dicts."""
    x = np.asarray(inputs["x"], dtype=np.float32)
    w_qkv = np.asarray(inputs["w_qkv"], dtype=np.float32)
    b_qkv = np.asarray(inputs["b_qkv"], dtype=np.float32)
    w_proj = np.asarray(inputs["w_proj"], dtype=np.float32)
    b_proj = np.asarray(inputs["b_proj"], dtype=np.float32)

    import ml_dtypes
    bf = ml_dtypes.bfloat16

    x_flat = x.reshape(-1, D)
    w_qT = np.ascontiguousarray((S_W * w_qkv[0:D]).T).astype(bf)
    w_kT = np.ascontiguousarray((S_W * w_qkv[D:2 * D]).T).astype(bf)
    w_vT = np.ascontiguousarray((S_W * w_qkv[2 * D:3 * D]).T).astype(bf)
    b_q = S_W * b_qkv[0:D]
    b_k = S_W * b_qkv[D:2 * D]
    b_v = b_qkv[2 * D:3 * D]
    w_pT = np.ascontiguousarray((S_W * w_proj).T).astype(bf)
    b_p_eff = OUT_SCALE * (b_proj + w_proj @ b_v)

    def bias_tile(b):
        return np.ascontiguousarray(b.reshape(NFT, P).T)

    shared = {
        "w_qT": w_qT, "w_kT": w_kT, "w_vT": w_vT, "w_pT": w_pT,
        "b_q": bias_tile(b_q), "b_k": bias_tile(b_k),
        "b_p": bias_tile(b_p_eff),
    }
    in_maps = []
    for i in range(NCORES):
        mine = x_flat[i * TQ:(i + 1) * TQ]
        partner = x_flat[(i ^ 1) * TQ:((i ^ 1) + 1) * TQ]
        xT_i = np.ascontiguousarray(
            np.concatenate([mine, partner], axis=0).T).astype(bf)
        in_maps.append({"xT": xT_i, **shared})
    return in_maps


def assemble_output(results, inputs):
    x = np.asarray(inputs["x"])
    y = np.empty((NCORES * TQ, D), dtype=np.float32)
    inv = 1.0 / OUT_SCALE
    for i in range(NCORES):
        y[i * TQ:(i + 1) * TQ] = results[i]["out"].T * inv
    return y.reshape(x.shape)


def run(inputs, trace=False, **kw):
    from concourse.bass_utils import run_bass_kernel_spmd
    key = "full"
    if key not in _COMPILED:
        _COMPILED[key] = build()
    nc = _COMPILED[key]
    in_maps = make_in_maps(inputs)
    res = run_bass_kernel_spmd(nc, in_maps, core_ids=list(range(NCORES)),
                               trace=trace, **kw)
    return res


def kernel(**inputs) -> np.ndarray:
    res = run(inputs, trace=False)
    return assemble_output(res.results, inputs)
